# revision 1
# baseline (speedup 1.0000x reference)
"""MiMoV2 MoE gate (moe_routing) on 8 Trainium2 NeuronCores.

Strategy:
  - Shard tokens (bsz*seq = 16384) across 8 cores, 2048 tokens each;
    replicate the [256, 4096] gate weight + bias.
  - Gating GEMM via an fp16 hi/lo split: x = x1 + x2/4096, W = w1 + w2/4096
    (cross terms pre-scaled by 2^12 so the low halves avoid fp16 subnormals).
    Three fp16 matmuls per chunk (x1*W1 into psum A; x1*w2 + x2*w1 into
    psum B, combined as A + B/4096): ~1e-7 rms logit error at 1 cyc/row
    fp16 PE speed (vs 4 cyc/row for native fp32).
  - sigmoid = 1/(1+exp(-x)) with the ~2-ULP Exp LUT + an accurate DVE
    reciprocal (the Sigmoid/Tanh LUTs have ~1e-6-scale error that flips
    near-tie expert choices).
  - Routing entirely on VectorE: per-group top-2 via segmented reduce_max +
    match_replace; top-4 groups via max8 threshold; group masking by adding
    (mask-1)*1e30 so allowed scores pass through bit-exact; top-8 pick via
    max8 + max_index; weights via a second masked max8 over raw sigmoid
    scores and an 8x8 index-match permute back into choice order.

Inputs (full):  hidden_states [4,4096,4096] f32, weight [256,4096] f32,
                e_score_correction_bias [256] f32
Output (full):  (topk_idx [16384,8] int32, topk_weight [16384,8] f32)
"""

import numpy as np

import concourse.tile as tile
from concourse import bacc, mybir
from concourse.bass_utils import run_bass_kernel_spmd

# problem shape (hardcoded per contract)
T_FULL = 16384
H = 4096
E = 256
G = 8
GS = E // G           # 32
TOPK = 8
SCALING = 2.5

N_CORES = 8
T_CORE = T_FULL // N_CORES    # 2048
# supertile token counts: small first (fast PE start), small last (short drain)
ST_SIZES = [128, 128] + [256] * 6 + [128, 128]
assert sum(ST_SIZES) == T_CORE
ST_OFFS = [sum(ST_SIZES[:i]) for i in range(len(ST_SIZES))]
ST_MAX = max(ST_SIZES)
N_CHUNK = H // 128            # 32 contraction chunks

_BUILT = None


CROSS_SCALE = 4096.0   # 2^12: cross terms are pre-scaled to dodge fp16 subnormals


def _build(trace=False):
    f32 = mybir.dt.float32
    f16 = mybir.dt.float16
    u32 = mybir.dt.uint32
    AF = mybir.ActivationFunctionType
    OP = mybir.AluOpType
    AX = mybir.AxisListType

    nc = bacc.Bacc("TRN2", target_bir_lowering=False, debug=False)

    # x = x1 + x2/CROSS_SCALE, W = w1 + w2/CROSS_SCALE (exact fp16 hi/lo
    # splits). Host pre-tiles into supertile-major layout so every DMA is
    # 128 long contiguous lines (fast descriptor-gen, full HBM efficiency).
    HC = N_CHUNK // 2
    x1 = nc.dram_tensor("x1", [128, N_CHUNK * T_CORE], f16, kind="ExternalInput").ap()
    x2 = nc.dram_tensor("x2", [128, N_CHUNK * T_CORE], f16, kind="ExternalInput").ap()
    w1 = nc.dram_tensor("w1", [2, 128, HC * E], f16, kind="ExternalInput").ap()
    w2 = nc.dram_tensor("w2", [2, 128, HC * E], f16, kind="ExternalInput").ap()
    bias_rep = nc.dram_tensor("bias_rep", [128, E], f32, kind="ExternalInput").ap()

    idx_out = nc.dram_tensor("idx_out", [T_CORE, TOPK], u32, kind="ExternalOutput").ap()
    w_out = nc.dram_tensor("w_out", [T_CORE, TOPK], f32, kind="ExternalOutput").ap()

    w1v = w1.rearrange("h p (c e) -> h p c e", c=HC)        # [2, 128, 16, 256]
    w2v = w2.rearrange("h p (c e) -> h p c e", c=HC)

    with tile.TileContext(nc) as tc:
        with tc.tile_pool(name="const", bufs=1) as cpool, \
             tc.tile_pool(name="xin", bufs=3) as xpool, \
             tc.tile_pool(name="mid", bufs=5) as mpool, \
             tc.tile_pool(name="small", bufs=6) as spool, \
             tc.tile_pool(name="psum", bufs=4, space="PSUM") as ppool:

            # W in chunk-quarters so the first matmuls only wait on ~1MB of
            # weights; bias DMA goes last (first used ~10us in)
            QC = HC // 2   # 8 chunks per quarter-tile
            Wt_ = {}
            for nm in ("W1", "W2"):
                for h in range(2):
                    for q in range(2):
                        wtile = cpool.tile([128, QC, E], f16, tag=f"{nm}{h}{q}")
                        Wt_[nm, 2 * h + q] = wtile
            BR = cpool.tile([128, E], f32, tag="BR")
            for q in range(4):
                nc.sync.dma_start(Wt_["W1", q][:],
                                  w1v[q // 2][:, (q % 2) * QC:(q % 2 + 1) * QC, :])
                nc.sync.dma_start(Wt_["W2", q][:],
                                  w2v[q // 2][:, (q % 2) * QC:(q % 2 + 1) * QC, :])
                if q == 0:
                    nc.sync.dma_start(BR[:], bias_rep)

            for st, (toff, tsz) in enumerate(zip(ST_OFFS, ST_SIZES)):
                xt1 = xpool.tile([128, N_CHUNK, ST_MAX], f16, tag="xt1")
                xt2 = xpool.tile([128, N_CHUNK, ST_MAX], f16, tag="xt2")
                seg = slice(N_CHUNK * toff, N_CHUNK * (toff + tsz))
                x1seg = x1[:, seg].rearrange("p (c t) -> p c t", c=N_CHUNK)
                x2seg = x2[:, seg].rearrange("p (c t) -> p c t", c=N_CHUNK)
                nc.sync.dma_start(xt1[:, :, 0:tsz], x1seg)
                nc.sync.dma_start(xt2[:, :, 0:tsz], x2seg)

                for sub in range(tsz // 128):
                    tok0 = toff + sub * 128
                    ps = ppool.tile([128, E], f32, tag="ps")     # x1*W1
                    ps2 = ppool.tile([128, E], f32, tag="ps2")   # x1*W2 + x2*W1
                    for c in range(N_CHUNK):
                        a1 = xt1[:, c, sub * 128:(sub + 1) * 128]
                        a2 = xt2[:, c, sub * 128:(sub + 1) * 128]
                        Wc1 = Wt_["W1", c // QC][:, c % QC, :]
                        Wc2 = Wt_["W2", c // QC][:, c % QC, :]
                        nc.tensor.matmul(ps[:], a1, Wc1,
                                         start=(c == 0), stop=(c == N_CHUNK - 1))
                        nc.tensor.matmul(ps2[:], a1, Wc2,
                                         start=(c == 0), stop=False)
                        nc.tensor.matmul(ps2[:], a2, Wc1,
                                         start=False, stop=(c == N_CHUNK - 1))

                    # -logits = -ps - ps2/CROSS_SCALE
                    t2n = mpool.tile([128, E], f32, tag="t2n")
                    nc.scalar.activation(t2n[:], ps2[:], AF.Copy,
                                         scale=-1.0 / CROSS_SCALE)
                    lgn = mpool.tile([128, E], f32, tag="lgn")
                    nc.vector.scalar_tensor_tensor(lgn[:], in0=ps[:], scalar=-1.0,
                                                   in1=t2n[:],
                                                   op0=OP.mult, op1=OP.add)

                    # sigmoid = 1/(1+exp(-x)); Exp LUT is ~2 ULP (vs 40-ULP
                    # budget Sigmoid/Tanh LUT whose error flips near-ties)
                    ex = mpool.tile([128, E], f32, tag="ex")
                    nc.scalar.activation(ex[:], lgn[:], AF.Exp)
                    ip1 = mpool.tile([128, E], f32, tag="ip1")
                    nc.scalar.activation(ip1[:], ex[:], AF.Copy, bias=1.0)
                    s_raw = mpool.tile([128, E], f32, tag="s_raw")
                    rscr = mpool.tile([128, E], f32, tag="rscr")
                    nc.vector.reciprocal_approx_accurate(s_raw[:], ip1[:], rscr[:])

                    # s_choice = sigmoid + bias
                    s_choice = mpool.tile([128, E], f32, tag="s_choice")
                    nc.vector.tensor_add(s_choice[:], s_raw[:], BR[:])
                    sc3 = s_choice[:].rearrange("p (g s) -> p g s", g=G)

                    # per-group top-2 sum
                    m1 = spool.tile([128, G], f32, tag="m1")
                    nc.vector.reduce_max(m1[:], sc3, axis=AX.X)
                    repl = mpool.tile([128, E], f32, tag="repl")
                    nc.vector.match_replace(repl[:], m1[:], s_choice[:], -1e30)
                    m2 = spool.tile([128, G], f32, tag="m2")
                    nc.vector.reduce_max(m2[:], repl[:].rearrange("p (g s) -> p g s", g=G),
                                         axis=AX.X)
                    gsum = spool.tile([128, G], f32, tag="gsum")
                    nc.vector.tensor_add(gsum[:], m1[:], m2[:])

                    # top-4 groups -> 0/1 mask -> +4.0 boost on allowed experts
                    gs8 = spool.tile([128, 8], f32, tag="gs8")
                    nc.vector.max(gs8[:], gsum[:])
                    gmask = spool.tile([128, G], f32, tag="gmask")
                    nc.vector.tensor_scalar(gmask[:], gsum[:], gs8[:, 3:4], None,
                                            op0=OP.is_ge)
                    # pen = (gmask-1)*1e30: exactly +0.0 for allowed groups, so
                    # allowed scores pass through BIT-EXACT (a +const boost
                    # would quantize them and flip near-ties)
                    pen = spool.tile([128, G], f32, tag="pen")
                    nc.vector.tensor_scalar(pen[:], gmask[:], 1.0, 1e30,
                                            op0=OP.subtract, op1=OP.mult)
                    s_mask = mpool.tile([128, E], f32, tag="s_mask")
                    pen_b = pen[:].unsqueeze(2).broadcast_to([128, G, GS])
                    nc.vector.tensor_tensor(
                        s_mask[:].rearrange("p (g s) -> p g s", g=G),
                        sc3, pen_b, op=OP.add)

                    # top-8 experts among allowed groups
                    v8 = spool.tile([128, 8], f32, tag="v8")
                    nc.vector.max(v8[:], s_mask[:])
                    i8 = spool.tile([128, 8], u32, tag="i8")
                    nc.vector.max_index(i8[:], v8[:], s_mask[:])
                    nc.sync.dma_start(idx_out[tok0:tok0 + 128, :], i8[:])

                    # raw scores of the selected 8 (same exact-passthrough mask)
                    sel = mpool.tile([128, E], f32, tag="sel")
                    nc.vector.tensor_scalar(sel[:], s_mask[:], v8[:, 7:8], None,
                                            op0=OP.is_ge)
                    penw = mpool.tile([128, E], f32, tag="penw")
                    nc.vector.tensor_scalar(penw[:], sel[:], 1.0, 1e30,
                                            op0=OP.subtract, op1=OP.mult)
                    r_sel = mpool.tile([128, E], f32, tag="r_sel")
                    nc.vector.tensor_add(r_sel[:], s_raw[:], penw[:])
                    w8d = spool.tile([128, 8], f32, tag="w8d")
                    nc.vector.max(w8d[:], r_sel[:])
                    ri8 = spool.tile([128, 8], u32, tag="ri8")
                    nc.vector.max_index(ri8[:], w8d[:], r_sel[:])

                    # permute w8d (raw-score order) into choice order
                    eq64 = spool.tile([128, 8, 8], f32, tag="eq64")
                    i8_b = i8[:].unsqueeze(2).broadcast_to([128, 8, 8])
                    ri8_b = ri8[:].unsqueeze(1).broadcast_to([128, 8, 8])
                    nc.vector.tensor_tensor(eq64[:], i8_b, ri8_b, op=OP.is_equal)
                    w64 = spool.tile([128, 8, 8], f32, tag="w64")
                    w8d_b = w8d[:].unsqueeze(1).broadcast_to([128, 8, 8])
                    nc.vector.tensor_tensor(w64[:], eq64[:], w8d_b, op=OP.mult)
                    w8p = spool.tile([128, 8], f32, tag="w8p")
                    nc.vector.reduce_sum(w8p[:], w64[:], axis=AX.X)

                    # w = w8p / sum(w8p) * 2.5
                    sum8 = spool.tile([128, 1], f32, tag="sum8")
                    nc.vector.reduce_sum(sum8[:], w8p[:], axis=AX.X)
                    den = spool.tile([128, 1], f32, tag="den")
                    nc.vector.tensor_scalar(den[:], sum8[:], 1.0 / SCALING, None,
                                            op0=OP.mult)
                    rcp = spool.tile([128, 1], f32, tag="rcp")
                    nc.vector.reciprocal(rcp[:], den[:])
                    wf = spool.tile([128, 8], f32, tag="wf")
                    nc.vector.tensor_scalar(wf[:], w8p[:], rcp[:, 0:1], None,
                                            op0=OP.mult)
                    nc.sync.dma_start(w_out[tok0:tok0 + 128, :], wf[:])

    nc.compile()
    return nc


def _get_built():
    global _BUILT
    if _BUILT is None:
        _BUILT = _build()
    return _BUILT


def _tile_x(arr):
    # [H, T_CORE] -> [128p, N_CHUNK*T_CORE]: per supertile segment (c-major,
    # token-minor) so each supertile DMA reads one contiguous span/partition
    v = arr.reshape(N_CHUNK, 128, T_CORE)
    segs = [np.ascontiguousarray(v[:, :, o:o + s].transpose(1, 0, 2)
                                 ).reshape(128, N_CHUNK * s)
            for o, s in zip(ST_OFFS, ST_SIZES)]
    return np.ascontiguousarray(np.concatenate(segs, axis=1))


def _tile_w(arr):
    # [H, E] -> [2, 128p, HC*E] with (p,c,e): arr[(h*HC+c)*128+p, e]
    HC = N_CHUNK // 2
    v = arr.reshape(2, HC, 128, E)
    return np.ascontiguousarray(v.transpose(0, 2, 1, 3).reshape(2, 128, HC * E))


def _prep_in_maps(hidden_states, weight, e_score_correction_bias):
    S = np.float32(CROSS_SCALE)
    x = np.asarray(hidden_states, dtype=np.float32).reshape(T_FULL, H)
    xT = np.ascontiguousarray(x.T)                      # [H, T]
    x1 = xT.astype(np.float16)
    x2 = ((xT - x1.astype(np.float32)) * S).astype(np.float16)

    W = np.asarray(weight, dtype=np.float32)
    Wt = np.ascontiguousarray(W.T)                      # [H, E]
    w1 = _tile_w(Wt.astype(np.float16))
    w2 = _tile_w(((Wt - Wt.astype(np.float16).astype(np.float32)) * S).astype(np.float16))

    b = np.asarray(e_score_correction_bias, dtype=np.float32)
    bias_rep = np.ascontiguousarray(np.tile(b[None, :], (128, 1)))

    in_maps = []
    for c in range(N_CORES):
        sl = slice(c * T_CORE, (c + 1) * T_CORE)
        in_maps.append({
            "x1": _tile_x(x1[:, sl]),
            "x2": _tile_x(x2[:, sl]),
            "w1": w1, "w2": w2, "bias_rep": bias_rep,
        })
    return in_maps


def kernel(hidden_states: np.ndarray, weight: np.ndarray,
           e_score_correction_bias: np.ndarray):
    in_maps = _prep_in_maps(hidden_states, weight, e_score_correction_bias)
    nc = _get_built()
    res = run_bass_kernel_spmd(nc, in_maps, list(range(N_CORES)))

    idx = np.concatenate([r["idx_out"] for r in res.results], axis=0).astype(np.int32)
    w = np.concatenate([r["w_out"] for r in res.results], axis=0).astype(np.float32)
    return idx, w



# revision 7
# speedup vs baseline: 1.1197x; 1.1197x over previous
"""MiMoV2 MoE gate (moe_routing) on 8 Trainium2 NeuronCores.

Strategy (v2):
  - Shard tokens (bsz*seq = 16384) across 8 cores, 2048 tokens each;
    replicate the [256, 4096] gate weight + bias.
  - Gating GEMM with W stationary and tokens moving (N=512), output
    [expert, token] in PSUM. Precision via fp16 main + ONE stacked
    fp8e4m3 DoubleRow correction pass:
      logits*2^17 = (x1*2^8)(W1*2^9)            [fp16, exact products]
                  + (dx*2^12)(W1*2^5)           [fp8 DR, chunk-paired]
                  + (x*2^-1)(dW*2^18)           [fp8 DR, chunk-paired]
    where x1 = fp16(x), dx = x - x1, W1 = fp16(W), dW = W - W1. All
    three pieces share one PSUM accumulation (scales match at 2^17), so
    no combine op is needed; the 2^-17 descale rides the psum->sbuf
    copy. Residual logit error ~1e-5 abs (vs fp16-single's 3.8e-4 which
    flips too many expert choices).
  - DoubleRow packs 2 contraction chunks per matmul (2 rows/PE cell),
    halving correction matmul time; its 256-col LDWEIGHTS is amortized
    by streaming 2 token-blocks per weight load.
  - PE transpose (identity matmul) returns logits to [token, expert];
    sigmoid+routing identical in spirit to v1: per-group top-2 via
    segmented reduce_max + match_replace; top-4 groups via max8
    threshold; exact-passthrough masking; top-8 via max8 + max_index;
    weights via masked max8 over raw scores + 8x8 index-match permute.

Inputs (full):  hidden_states [4,4096,4096] f32, weight [256,4096] f32,
                e_score_correction_bias [256] f32
Output (full):  (topk_idx [16384,8] int32, topk_weight [16384,8] f32)
"""

import numpy as np
import ml_dtypes

import concourse.tile as tile
from concourse import bacc, mybir
from concourse.bass_utils import run_bass_kernel_spmd

# problem shape (hardcoded per contract)
T_FULL = 16384
H = 4096
E = 256
G = 8
GS = E // G           # 32
TOPK = 8
SCALING = 2.5

N_CORES = 8
T_CORE = T_FULL // N_CORES    # 2048
NCH = H // 128                # 32 contraction chunks
NQ = NCH // 2                 # 16 chunk-pairs for DoubleRow
TB = 512                      # token block (psum bank = 512 f32)
NB = T_CORE // TB             # 4 blocks
NSUB = TB // 128              # 4 token subtiles per block

SC_MAIN = 2.0 ** 17           # psum scale
S_X1 = 2.0 ** 8               # x1 pre-scale (x1*W1 -> 2^17)
S_W1 = 2.0 ** 9
S_DX = 2.0 ** 12              # dx pre-scale (dx*W1 -> 2^17)
S_W1_8 = 2.0 ** 5
S_XC = 2.0 ** -1              # coarse-x pre-scale (x*dW -> 2^17)
S_DW = 2.0 ** 18

_BUILT = None


def _build():
    f32 = mybir.dt.float32
    f16 = mybir.dt.float16
    f8 = mybir.dt.float8e4
    u32 = mybir.dt.uint32
    AF = mybir.ActivationFunctionType
    OP = mybir.AluOpType
    AX = mybir.AxisListType
    DR = mybir.MatmulPerfMode.DoubleRow

    nc = bacc.Bacc("TRN2", target_bir_lowering=False, debug=False)

    # x arrays, contraction on partitions, block/chunk/token free layout
    x1 = nc.dram_tensor("x1", [NB, 128, NCH, TB], f16, kind="ExternalInput").ap()
    dx8 = nc.dram_tensor("dx8", [NB, 128, NCH, TB], f8, kind="ExternalInput").ap()
    xc8 = nc.dram_tensor("xc8", [NB, 128, NCH, TB], f8, kind="ExternalInput").ap()
    # W arrays: [128, chunk, ehalf, 128e]
    w1 = nc.dram_tensor("w1", [128, NCH, 2, 128], f16, kind="ExternalInput").ap()
    w18 = nc.dram_tensor("w18", [128, NCH, 2, 128], f8, kind="ExternalInput").ap()
    dw8 = nc.dram_tensor("dw8", [128, NCH, 2, 128], f8, kind="ExternalInput").ap()
    bias_rep = nc.dram_tensor("bias_rep", [128, E], f32, kind="ExternalInput").ap()
    id_in = nc.dram_tensor("id_in", [128, 128], f32, kind="ExternalInput").ap()

    idx_out = nc.dram_tensor("idx_out", [T_CORE, TOPK], u32, kind="ExternalOutput").ap()
    w_out = nc.dram_tensor("w_out", [T_CORE, TOPK], f32, kind="ExternalOutput").ap()

    with tile.TileContext(nc) as tc:
        with tc.tile_pool(name="const", bufs=1) as cpool, \
             tc.tile_pool(name="xin", bufs=1) as xpool, \
             tc.tile_pool(name="comb", bufs=3) as kpool, \
             tc.tile_pool(name="mid", bufs=4) as mpool, \
             tc.tile_pool(name="small", bufs=8) as spool, \
             tc.tile_pool(name="pacc", bufs=1, space="PSUM") as papool, \
             tc.tile_pool(name="ptr", bufs=4, space="PSUM") as ptpool:

            # constants: W tiles (chunk-quartered DMA so first MMs start
            # early), bias, identity
            W1t = cpool.tile([128, NCH, 2, 128], f16, tag="W1t")
            W18t = cpool.tile([128, NCH, 2, 128], f8, tag="W18t")
            dW8t = cpool.tile([128, NCH, 2, 128], f8, tag="dW8t")
            BR = cpool.tile([128, E], f32, tag="BR")
            IDT = cpool.tile([128, 128], f32, tag="IDT")
            QC = NCH // 4
            for q in range(4):
                sl = slice(q * QC, (q + 1) * QC)
                nc.sync.dma_start(W18t[:, sl, :, :], w18[:, sl, :, :])
                nc.sync.dma_start(dW8t[:, sl, :, :], dw8[:, sl, :, :])
                nc.sync.dma_start(W1t[:, sl, :, :], w1[:, sl, :, :])
                if q == 0:
                    nc.sync.dma_start(IDT[:], id_in)
                    nc.sync.dma_start(BR[:], bias_rep)

            for bp in range(NB // 2):     # block pairs
                b0, b1 = 2 * bp, 2 * bp + 1
                xt1, xd8, xc8t = {}, {}, {}
                for b in (b0, b1):
                    xd8[b] = xpool.tile([128, NCH, TB], f8, tag=f"dx_{b % 2}", name=f"xd8_{b % 2}")
                    xc8t[b] = xpool.tile([128, NCH, TB], f8, tag=f"xc_{b % 2}", name=f"xc8t_{b % 2}")
                    xt1[b] = xpool.tile([128, NCH, TB], f16, tag=f"x1_{b % 2}", name=f"xt1_{b % 2}")
                    nc.sync.dma_start(xd8[b][:], dx8[b])
                    nc.sync.dma_start(xc8t[b][:], xc8[b])
                    nc.sync.dma_start(xt1[b][:], x1[b])

                ps = {}
                for h in range(2):
                    for b in (b0, b1):
                        ps[h, b] = papool.tile([128, TB], f32, tag=f"ps{h}{b % 2}", name=f"ps_{h}_{b % 2}")
                # corr fp8 DoubleRow: chunk-paired; 2 blocks per weight load
                # (DR LDWEIGHTS is 256 cols; 2 N=512 streams just cover it)
                for h in range(2):
                    for q in range(NQ):
                        wsl = W18t[:, 2 * q:2 * q + 2, h, :]
                        for b in (b0, b1):
                            nc.tensor.matmul(ps[h, b][:], wsl,
                                             xd8[b][:, 2 * q:2 * q + 2, :],
                                             perf_mode=DR, start=(q == 0),
                                             stop=False)
                for h in range(2):
                    for q in range(NQ):
                        wsl = dW8t[:, 2 * q:2 * q + 2, h, :]
                        for b in (b0, b1):
                            nc.tensor.matmul(ps[h, b][:], wsl,
                                             xc8t[b][:, 2 * q:2 * q + 2, :],
                                             perf_mode=DR, start=False, stop=False)
                # main fp16: b-outer so x1[b0] frees early for prefetch
                # (fp16 LDWEIGHTS of 128 cols hides under the N=512 stream)
                for b in (b0, b1):
                    for h in range(2):
                        for c in range(NCH):
                            nc.tensor.matmul(ps[h, b][:], W1t[:, c, h, :],
                                             xt1[b][:, c, :],
                                             start=False, stop=(c == NCH - 1))

                for b in (b0, b1):
                    # descale to logits while copying psum->sbuf
                    cb = {}
                    for h in range(2):
                        cb[h] = kpool.tile([128, TB], f32, tag=f"cb{h}", name=f"cb_{h}")
                        nc.scalar.activation(cb[h][:], ps[h, b][:], AF.Copy,
                                             scale=1.0 / SC_MAIN)
                    for g in range(NSUB):
                        tok0 = b * TB + g * 128
                        pt = ptpool.tile([128, E], f32, tag="pt")
                        for h in range(2):
                            nc.tensor.transpose(pt[:, h * 128:(h + 1) * 128],
                                                cb[h][:, g * 128:(g + 1) * 128],
                                                IDT[:])

                        # ---- routing for 128 tokens ----
                        s_raw = mpool.tile([128, E], f32, tag="s_raw")
                        nc.scalar.activation(s_raw[:], pt[:], AF.Sigmoid)
                        s_choice = mpool.tile([128, E], f32, tag="s_choice")
                        nc.gpsimd.tensor_add(s_choice[:], s_raw[:], BR[:])
                        sc3 = s_choice[:].rearrange("p (g s) -> p g s", g=G)

                        # per-group top-2 sum
                        m1 = spool.tile([128, G], f32, tag="m1")
                        nc.vector.reduce_max(m1[:], sc3, axis=AX.X)
                        repl = mpool.tile([128, E], f32, tag="repl")
                        nc.vector.match_replace(repl[:], m1[:], s_choice[:], -1e30)
                        m2 = spool.tile([128, G], f32, tag="m2")
                        nc.vector.reduce_max(
                            m2[:], repl[:].rearrange("p (g s) -> p g s", g=G),
                            axis=AX.X)
                        gsum = spool.tile([128, G], f32, tag="gsum")
                        nc.vector.tensor_add(gsum[:], m1[:], m2[:])

                        # top-4 groups -> pen: 0.0 for allowed, -1e30 else
                        gs8 = spool.tile([128, 8], f32, tag="gs8")
                        nc.vector.max(gs8[:], gsum[:])
                        pen = spool.tile([128, G], f32, tag="pen")
                        nc.vector.tensor_scalar(pen[:], gsum[:], gs8[:, 3:4],
                                                -1e30, op0=OP.is_lt, op1=OP.mult)
                        s_mask = mpool.tile([128, E], f32, tag="s_mask")
                        pen_b = pen[:].unsqueeze(2).broadcast_to([128, G, GS])
                        nc.vector.tensor_tensor(
                            s_mask[:].rearrange("p (g s) -> p g s", g=G),
                            sc3, pen_b, op=OP.add)

                        # top-8 experts among allowed groups
                        v8 = spool.tile([128, 8], f32, tag="v8")
                        nc.vector.max(v8[:], s_mask[:])
                        i8 = spool.tile([128, 8], u32, tag="i8")
                        nc.vector.max_index(i8[:], v8[:], s_mask[:])
                        nc.sync.dma_start(idx_out[tok0:tok0 + 128, :], i8[:])

                        # raw scores of the selected 8 (zero elsewhere);
                        # selected raw scores are > 0 so max8 finds them
                        r_sel = mpool.tile([128, E], f32, tag="r_sel")
                        nc.vector.scalar_tensor_tensor(
                            r_sel[:], in0=s_mask[:], scalar=v8[:, 7:8],
                            in1=s_raw[:], op0=OP.is_ge, op1=OP.mult)
                        w8d = spool.tile([128, 8], f32, tag="w8d")
                        nc.vector.max(w8d[:], r_sel[:])
                        ri8 = spool.tile([128, 8], u32, tag="ri8")
                        nc.vector.max_index(ri8[:], w8d[:], r_sel[:])

                        # permute w8d (raw order) into choice order via 8x8
                        # index match
                        eq64 = spool.tile([128, 8, 8], f32, tag="eq64")
                        i8_b = i8[:].unsqueeze(2).broadcast_to([128, 8, 8])
                        ri8_b = ri8[:].unsqueeze(1).broadcast_to([128, 8, 8])
                        nc.vector.tensor_tensor(eq64[:], i8_b, ri8_b,
                                                op=OP.is_equal)
                        w64 = spool.tile([128, 8, 8], f32, tag="w64")
                        w8d_b = w8d[:].unsqueeze(1).broadcast_to([128, 8, 8])
                        nc.vector.tensor_tensor(w64[:], eq64[:], w8d_b,
                                                op=OP.mult)
                        w8p = spool.tile([128, 8], f32, tag="w8p")
                        nc.vector.reduce_sum(w8p[:], w64[:], axis=AX.X)

                        # wf = w8p / sum(w8d) * 2.5
                        sum8 = spool.tile([128, 1], f32, tag="sum8")
                        nc.vector.reduce_sum(sum8[:], w8d[:], axis=AX.X)
                        rcp = spool.tile([128, 1], f32, tag="rcp")
                        nc.vector.reciprocal(rcp[:], sum8[:])
                        wf = spool.tile([128, 8], f32, tag="wf")
                        nc.vector.tensor_scalar(wf[:], w8p[:], rcp[:, 0:1],
                                                SCALING, op0=OP.mult, op1=OP.mult)
                        nc.sync.dma_start(w_out[tok0:tok0 + 128, :], wf[:])

    nc.compile()
    return nc


def _get_built():
    global _BUILT
    if _BUILT is None:
        _BUILT = _build()
    return _BUILT


def _part(a, inner):
    # [H, inner] -> [128, NCH, inner] with element (p, c, i) = a[c*128+p, i]
    return np.ascontiguousarray(a.reshape(NCH, 128, inner).transpose(1, 0, 2))


def _prep_in_maps(hidden_states, weight, e_score_correction_bias):
    f8 = ml_dtypes.float8_e4m3
    x = np.asarray(hidden_states, dtype=np.float32).reshape(T_FULL, H)
    xT = np.ascontiguousarray(x.T)                      # [H, T]
    x1f = xT.astype(np.float16)
    dx = xT - x1f.astype(np.float32)

    x1s = (x1f.astype(np.float32) * S_X1).astype(np.float16)   # exact scale
    dx8f = (dx * S_DX).astype(f8)
    xc8f = (xT * S_XC).astype(f8)

    W = np.asarray(weight, dtype=np.float32)
    Wt = np.ascontiguousarray(W.T)                      # [H, E]
    W1f = Wt.astype(np.float16)
    dW = Wt - W1f.astype(np.float32)
    w1h = _part((W1f.astype(np.float32) * S_W1).astype(np.float16), E)
    w18h = _part((W1f.astype(np.float32) * S_W1_8).astype(f8), E)
    dw8h = _part((dW * S_DW).astype(f8), E)
    w1h = w1h.reshape(128, NCH, 2, 128)
    w18h = w18h.reshape(128, NCH, 2, 128)
    dw8h = dw8h.reshape(128, NCH, 2, 128)

    b = np.asarray(e_score_correction_bias, dtype=np.float32)
    bias_rep = np.ascontiguousarray(np.tile(b[None, :], (128, 1)))
    ident = np.eye(128, dtype=np.float32)

    def blocks(a):
        # [128, NCH, T_CORE] -> [NB, 128, NCH, TB]
        v = a.reshape(128, NCH, NB, TB)
        return np.ascontiguousarray(v.transpose(2, 0, 1, 3))

    in_maps = []
    for c in range(N_CORES):
        sl = slice(c * T_CORE, (c + 1) * T_CORE)
        in_maps.append({
            "x1": blocks(_part(x1s[:, sl], T_CORE)),
            "dx8": blocks(_part(dx8f[:, sl], T_CORE)),
            "xc8": blocks(_part(xc8f[:, sl], T_CORE)),
            "w1": w1h, "w18": w18h, "dw8": dw8h,
            "bias_rep": bias_rep, "id_in": ident,
        })
    return in_maps


def kernel(hidden_states: np.ndarray, weight: np.ndarray,
           e_score_correction_bias: np.ndarray):
    in_maps = _prep_in_maps(hidden_states, weight, e_score_correction_bias)
    nc = _get_built()
    res = run_bass_kernel_spmd(nc, in_maps, list(range(N_CORES)))

    idx = np.concatenate([r["idx_out"] for r in res.results], axis=0).astype(np.int32)
    w = np.concatenate([r["w_out"] for r in res.results], axis=0).astype(np.float32)
    return idx, w


# revision 10
# speedup vs baseline: 1.1246x; 1.0044x over previous
"""MiMoV2 MoE gate (moe_routing) on 8 Trainium2 NeuronCores.

Strategy (v2):
  - Shard tokens (bsz*seq = 16384) across 8 cores, 2048 tokens each;
    replicate the [256, 4096] gate weight + bias.
  - Gating GEMM with W stationary and tokens moving (N=512), output
    [expert, token] in PSUM. Precision via fp16 main + ONE stacked
    fp8e4m3 DoubleRow correction pass:
      logits*2^17 = (x1*2^8)(W1*2^9)            [fp16, exact products]
                  + (dx*2^12)(W1*2^5)           [fp8 DR, chunk-paired]
                  + (x*2^-1)(dW*2^18)           [fp8 DR, chunk-paired]
    where x1 = fp16(x), dx = x - x1, W1 = fp16(W), dW = W - W1. All
    three pieces share one PSUM accumulation (scales match at 2^17), so
    no combine op is needed; the 2^-17 descale rides the psum->sbuf
    copy. Residual logit error ~1e-5 abs (vs fp16-single's 3.8e-4 which
    flips too many expert choices).
  - DoubleRow packs 2 contraction chunks per matmul (2 rows/PE cell),
    halving correction matmul time; its 256-col LDWEIGHTS is amortized
    by streaming 2 token-blocks per weight load.
  - PE transpose (identity matmul) returns logits to [token, expert];
    sigmoid+routing identical in spirit to v1: per-group top-2 via
    segmented reduce_max + match_replace; top-4 groups via max8
    threshold; exact-passthrough masking; top-8 via max8 + max_index;
    weights via masked max8 over raw scores + 8x8 index-match permute.

Inputs (full):  hidden_states [4,4096,4096] f32, weight [256,4096] f32,
                e_score_correction_bias [256] f32
Output (full):  (topk_idx [16384,8] int32, topk_weight [16384,8] f32)
"""

import numpy as np
import ml_dtypes

import concourse.tile as tile
from concourse import bacc, mybir
from concourse.bass_utils import run_bass_kernel_spmd

# problem shape (hardcoded per contract)
T_FULL = 16384
H = 4096
E = 256
G = 8
GS = E // G           # 32
TOPK = 8
SCALING = 2.5

N_CORES = 8
T_CORE = T_FULL // N_CORES    # 2048
NCH = H // 128                # 32 contraction chunks
NQ = NCH // 2                 # 16 chunk-pairs for DoubleRow
TB = 512                      # token block (psum bank = 512 f32)
NB = T_CORE // TB             # 4 blocks
NSUB = TB // 128              # 4 token subtiles per block

SC_MAIN = 2.0 ** 17           # psum scale
S_X1 = 2.0 ** 8               # x1 pre-scale (x1*W1 -> 2^17)
S_W1 = 2.0 ** 9
S_DX = 2.0 ** 12              # dx pre-scale (dx*W1 -> 2^17)
S_W1_8 = 2.0 ** 5
S_XC = 2.0 ** -1              # coarse-x pre-scale (x*dW -> 2^17)
S_DW = 2.0 ** 18

_BUILT = None


def _build():
    f32 = mybir.dt.float32
    f16 = mybir.dt.float16
    f8 = mybir.dt.float8e4
    u32 = mybir.dt.uint32
    AF = mybir.ActivationFunctionType
    OP = mybir.AluOpType
    AX = mybir.AxisListType
    DR = mybir.MatmulPerfMode.DoubleRow

    nc = bacc.Bacc("TRN2", target_bir_lowering=False, debug=False)

    # x arrays, contraction on partitions, block/chunk/token free layout
    x1 = nc.dram_tensor("x1", [NB, 128, NCH, TB], f16, kind="ExternalInput").ap()
    dx8 = nc.dram_tensor("dx8", [NB, 128, NCH, TB], f8, kind="ExternalInput").ap()
    xc8 = nc.dram_tensor("xc8", [NB, 128, NCH, TB], f8, kind="ExternalInput").ap()
    # W arrays: [128, chunk, ehalf, 128e]
    w1 = nc.dram_tensor("w1", [128, NCH, 2, 128], f16, kind="ExternalInput").ap()
    w18 = nc.dram_tensor("w18", [128, NCH, 2, 128], f8, kind="ExternalInput").ap()
    dw8 = nc.dram_tensor("dw8", [128, NCH, 2, 128], f8, kind="ExternalInput").ap()
    bias_rep = nc.dram_tensor("bias_rep", [128, E], f32, kind="ExternalInput").ap()
    id_in = nc.dram_tensor("id_in", [128, 128], f32, kind="ExternalInput").ap()

    idx_out = nc.dram_tensor("idx_out", [T_CORE, TOPK], u32, kind="ExternalOutput").ap()
    w_out = nc.dram_tensor("w_out", [T_CORE, TOPK], f32, kind="ExternalOutput").ap()

    with tile.TileContext(nc) as tc:
        with tc.tile_pool(name="const", bufs=1) as cpool, \
             tc.tile_pool(name="xin", bufs=1) as xpool, \
             tc.tile_pool(name="comb", bufs=3) as kpool, \
             tc.tile_pool(name="mid", bufs=4) as mpool, \
             tc.tile_pool(name="small", bufs=8) as spool, \
             tc.tile_pool(name="pacc", bufs=2, space="PSUM") as papool, \
             tc.tile_pool(name="ptr", bufs=4, space="PSUM") as ptpool:

            # constants: W tiles (chunk-quartered DMA so first MMs start
            # early), bias, identity
            W1t = cpool.tile([128, NCH, 2, 128], f16, tag="W1t")
            W18t = cpool.tile([128, NCH, 2, 128], f8, tag="W18t")
            dW8t = cpool.tile([128, NCH, 2, 128], f8, tag="dW8t")
            BR = cpool.tile([128, E], f32, tag="BR")
            IDT = cpool.tile([128, 128], f32, tag="IDT")
            QC = NCH // 4
            for q in range(4):
                sl = slice(q * QC, (q + 1) * QC)
                nc.sync.dma_start(W18t[:, sl, :, :], w18[:, sl, :, :])
                nc.sync.dma_start(dW8t[:, sl, :, :], dw8[:, sl, :, :])
                nc.sync.dma_start(W1t[:, sl, :, :], w1[:, sl, :, :])
                if q == 0:
                    nc.sync.dma_start(IDT[:], id_in)
                    nc.sync.dma_start(BR[:], bias_rep)

            for b in range(NB):           # 512-token blocks
                # halved DMAs so the first matmuls start ~6us in
                xt1 = xpool.tile([128, NCH, TB], f16, tag=f"x1_{b % 2}", name=f"xt1_{b % 2}")
                xd8 = xpool.tile([128, NCH, TB], f8, tag=f"dx_{b % 2}", name=f"xd8_{b % 2}")
                xc8t = xpool.tile([128, NCH, TB], f8, tag=f"xc_{b % 2}", name=f"xc8t_{b % 2}")
                HC = NCH // 2
                for hf in range(2):
                    sl = slice(hf * HC, (hf + 1) * HC)
                    nc.sync.dma_start(xt1[:, sl, :], x1[b][:, sl, :])
                    nc.sync.dma_start(xd8[:, sl, :], dx8[b][:, sl, :])
                    nc.sync.dma_start(xc8t[:, sl, :], xc8[b][:, sl, :])

                # 1:1 interleave of main fp16 MMs (N=512 stream, 213ns) and
                # fp8 DoubleRow corr MMs: each DR LDWEIGHTS (256 cols,
                # ~213ns) prefetches into the background weight buffer
                # during the preceding main MM's stream.
                ps = {}
                for h in range(2):
                    ps[h] = papool.tile([128, TB], f32, tag=f"ps{h}", name=f"ps_{h}")
                    for g in range(NCH):
                        nc.tensor.matmul(ps[h][:], W1t[:, g, h, :],
                                         xt1[:, g, :],
                                         start=(g == 0), stop=False)
                        q = g // 2
                        if g % 2 == 0:
                            nc.tensor.matmul(ps[h][:],
                                             W18t[:, 2 * q:2 * q + 2, h, :],
                                             xd8[:, 2 * q:2 * q + 2, :],
                                             perf_mode=DR, start=False, stop=False)
                        else:
                            nc.tensor.matmul(ps[h][:],
                                             dW8t[:, 2 * q:2 * q + 2, h, :],
                                             xc8t[:, 2 * q:2 * q + 2, :],
                                             perf_mode=DR, start=False,
                                             stop=(g == NCH - 1))

                if True:
                    # descale to logits while copying psum->sbuf
                    cb = {}
                    for h in range(2):
                        cb[h] = kpool.tile([128, TB], f32, tag=f"cb{h}", name=f"cb_{h}")
                        nc.scalar.activation(cb[h][:], ps[h][:], AF.Copy,
                                             scale=1.0 / SC_MAIN)
                    for g in range(NSUB):
                        tok0 = b * TB + g * 128
                        pt = ptpool.tile([128, E], f32, tag="pt")
                        for h in range(2):
                            nc.tensor.transpose(pt[:, h * 128:(h + 1) * 128],
                                                cb[h][:, g * 128:(g + 1) * 128],
                                                IDT[:])

                        # ---- routing for 128 tokens ----
                        s_raw = mpool.tile([128, E], f32, tag="s_raw")
                        nc.scalar.activation(s_raw[:], pt[:], AF.Sigmoid)
                        s_choice = mpool.tile([128, E], f32, tag="s_choice")
                        nc.vector.tensor_add(s_choice[:], s_raw[:], BR[:])
                        sc3 = s_choice[:].rearrange("p (g s) -> p g s", g=G)

                        # per-group top-2 sum
                        m1 = spool.tile([128, G], f32, tag="m1")
                        nc.vector.reduce_max(m1[:], sc3, axis=AX.X)
                        repl = mpool.tile([128, E], f32, tag="repl")
                        nc.vector.match_replace(repl[:], m1[:], s_choice[:], -1e30)
                        m2 = spool.tile([128, G], f32, tag="m2")
                        nc.vector.reduce_max(
                            m2[:], repl[:].rearrange("p (g s) -> p g s", g=G),
                            axis=AX.X)
                        gsum = spool.tile([128, G], f32, tag="gsum")
                        nc.vector.tensor_add(gsum[:], m1[:], m2[:])

                        # top-4 groups -> pen: 0.0 for allowed, -1e30 else
                        gs8 = spool.tile([128, 8], f32, tag="gs8")
                        nc.vector.max(gs8[:], gsum[:])
                        pen = spool.tile([128, G], f32, tag="pen")
                        nc.vector.tensor_scalar(pen[:], gsum[:], gs8[:, 3:4],
                                                -1e30, op0=OP.is_lt, op1=OP.mult)
                        s_mask = mpool.tile([128, E], f32, tag="s_mask")
                        pen_b = pen[:].unsqueeze(2).broadcast_to([128, G, GS])
                        nc.vector.tensor_tensor(
                            s_mask[:].rearrange("p (g s) -> p g s", g=G),
                            sc3, pen_b, op=OP.add)

                        # top-8 experts among allowed groups
                        v8 = spool.tile([128, 8], f32, tag="v8")
                        nc.vector.max(v8[:], s_mask[:])
                        i8 = spool.tile([128, 8], u32, tag="i8")
                        nc.vector.max_index(i8[:], v8[:], s_mask[:])
                        nc.sync.dma_start(idx_out[tok0:tok0 + 128, :], i8[:])

                        # raw scores of the selected 8 (zero elsewhere);
                        # selected raw scores are > 0 so max8 finds them
                        r_sel = mpool.tile([128, E], f32, tag="r_sel")
                        nc.vector.scalar_tensor_tensor(
                            r_sel[:], in0=s_mask[:], scalar=v8[:, 7:8],
                            in1=s_raw[:], op0=OP.is_ge, op1=OP.mult)
                        w8d = spool.tile([128, 8], f32, tag="w8d")
                        nc.vector.max(w8d[:], r_sel[:])
                        ri8 = spool.tile([128, 8], u32, tag="ri8")
                        nc.vector.max_index(ri8[:], w8d[:], r_sel[:])

                        # permute w8d (raw order) into choice order via 8x8
                        # index match
                        eq64 = spool.tile([128, 8, 8], f32, tag="eq64")
                        i8_b = i8[:].unsqueeze(2).broadcast_to([128, 8, 8])
                        ri8_b = ri8[:].unsqueeze(1).broadcast_to([128, 8, 8])
                        nc.vector.tensor_tensor(eq64[:], i8_b, ri8_b,
                                                op=OP.is_equal)
                        w64 = spool.tile([128, 8, 8], f32, tag="w64")
                        w8d_b = w8d[:].unsqueeze(1).broadcast_to([128, 8, 8])
                        nc.vector.tensor_tensor(w64[:], eq64[:], w8d_b,
                                                op=OP.mult)
                        w8p = spool.tile([128, 8], f32, tag="w8p")
                        nc.vector.reduce_sum(w8p[:], w64[:], axis=AX.X)

                        # wf = w8p / sum(w8d) * 2.5
                        sum8 = spool.tile([128, 1], f32, tag="sum8")
                        nc.vector.reduce_sum(sum8[:], w8d[:], axis=AX.X)
                        rcp = spool.tile([128, 1], f32, tag="rcp")
                        nc.vector.reciprocal(rcp[:], sum8[:])
                        wf = spool.tile([128, 8], f32, tag="wf")
                        nc.vector.tensor_scalar(wf[:], w8p[:], rcp[:, 0:1],
                                                SCALING, op0=OP.mult, op1=OP.mult)
                        nc.sync.dma_start(w_out[tok0:tok0 + 128, :], wf[:])

    nc.compile()
    return nc


def _get_built():
    global _BUILT
    if _BUILT is None:
        _BUILT = _build()
    return _BUILT


def _part(a, inner):
    # [H, inner] -> [128, NCH, inner] with element (p, c, i) = a[c*128+p, i]
    return np.ascontiguousarray(a.reshape(NCH, 128, inner).transpose(1, 0, 2))


def _prep_in_maps(hidden_states, weight, e_score_correction_bias):
    f8 = ml_dtypes.float8_e4m3
    x = np.asarray(hidden_states, dtype=np.float32).reshape(T_FULL, H)
    xT = np.ascontiguousarray(x.T)                      # [H, T]
    x1f = xT.astype(np.float16)
    dx = xT - x1f.astype(np.float32)

    x1s = (x1f.astype(np.float32) * S_X1).astype(np.float16)   # exact scale
    dx8f = (dx * S_DX).astype(f8)
    xc8f = (xT * S_XC).astype(f8)

    W = np.asarray(weight, dtype=np.float32)
    Wt = np.ascontiguousarray(W.T)                      # [H, E]
    W1f = Wt.astype(np.float16)
    dW = Wt - W1f.astype(np.float32)
    w1h = _part((W1f.astype(np.float32) * S_W1).astype(np.float16), E)
    w18h = _part((W1f.astype(np.float32) * S_W1_8).astype(f8), E)
    dw8h = _part((dW * S_DW).astype(f8), E)
    w1h = w1h.reshape(128, NCH, 2, 128)
    w18h = w18h.reshape(128, NCH, 2, 128)
    dw8h = dw8h.reshape(128, NCH, 2, 128)

    b = np.asarray(e_score_correction_bias, dtype=np.float32)
    bias_rep = np.ascontiguousarray(np.tile(b[None, :], (128, 1)))
    ident = np.eye(128, dtype=np.float32)

    def blocks(a):
        # [128, NCH, T_CORE] -> [NB, 128, NCH, TB]
        v = a.reshape(128, NCH, NB, TB)
        return np.ascontiguousarray(v.transpose(2, 0, 1, 3))

    in_maps = []
    for c in range(N_CORES):
        sl = slice(c * T_CORE, (c + 1) * T_CORE)
        in_maps.append({
            "x1": blocks(_part(x1s[:, sl], T_CORE)),
            "dx8": blocks(_part(dx8f[:, sl], T_CORE)),
            "xc8": blocks(_part(xc8f[:, sl], T_CORE)),
            "w1": w1h, "w18": w18h, "dw8": dw8h,
            "bias_rep": bias_rep, "id_in": ident,
        })
    return in_maps


def kernel(hidden_states: np.ndarray, weight: np.ndarray,
           e_score_correction_bias: np.ndarray):
    in_maps = _prep_in_maps(hidden_states, weight, e_score_correction_bias)
    nc = _get_built()
    res = run_bass_kernel_spmd(nc, in_maps, list(range(N_CORES)))

    idx = np.concatenate([r["idx_out"] for r in res.results], axis=0).astype(np.int32)
    w = np.concatenate([r["w_out"] for r in res.results], axis=0).astype(np.float32)
    return idx, w
